# revision 18
# baseline (speedup 1.0000x reference)
"""Memory-efficient Dice loss on 8 Trainium2 NeuronCores.

Full inputs:
  logits  (2, 16, 64, 128, 128) fp32
  targets (2, 64, 128, 128) int64  (values 0..15)
Output: scalar fp32 loss = 1 - mean_{b, c != 0} dice[b, c].

Sharding: 8 cores over (B=2) x (D quartered into 4 slabs of 16).

Key trick: the HOST class-sorts the voxels of each core shard so that
every 128-voxel chunk is single-class, with a fixed compile-time
layout: class c owns chunks [128c, 128c+120) ("live"); the remaining
8 chunks per class region hold the leftover voxels ("dead", computed
on the host, ~6%). Voxel order is irrelevant to the dice statistics.

Device math per chunk j (128 voxels on partitions, chunk-major
[p, (j, c)] bf16 layout):
  e = exp(logits); Z = pair-tree sum over the 16-class runs;
  r = 1/Z;  S[c', class(j)] += sum_p e[p, j, c'] * r[p, j]
One packed matmul per 8-chunk slab: lhsT = E 128-col slab, rhs = the
raw 8 r-columns of that slab, out -> the class's private 8-column
PSUM region, accumulated over the class's 15 slabs. The one-hot never
exists: it is encoded in the output-region assignment; the host
extracts S[c', c] = sum_j out[j*16 + c', c*8 + j] from the final
128x128 matrix (intersection = diag, probs_sum = row sums).

Host handles: the sort/permutation, softmax of dead-chunk voxels,
exact bincount counts, and the final dice formula. If a class has
fewer than 15360 voxels its live region is padded with zero-logit
dummies whose exact uniform contribution (1/16 per entry) is
subtracted analytically.

Engine budget per core (measured rates): DMA 23us (8 MB bf16, 4
16KiB-descriptor DMAs), ACT exp 27us, DVE pair-tree+recip ~22us with
a GPSIMD level-1 share ~17us, PE 240 packed matmuls ~22us.
"""

import numpy as np
import ml_dtypes

import concourse.mybir as mybir
import concourse.tile as tile
from concourse import bacc
from concourse.bass_utils import run_bass_kernel_spmd

B, C, D, H, W = 2, 16, 64, 128, 128
P = 128            # SBUF partitions
NCORES = 8
DSH = D // 4       # d-planes per core
N = DSH * H * W    # voxels per core = 262144
NCHUNK = N // P    # 2048 chunks of 128 voxels
NBLK = 8
T = NCHUNK // NBLK # 256 chunks per block
CREG = 128         # chunks per class region
LIVE_SLABS = 15    # live slabs per class; the 16th is dead
SLAB = 8           # chunks per matmul
LIVE = LIVE_SLABS * SLAB       # 120 live chunks per class
NLIVE_C = LIVE * P             # 15360 live voxels per class

SMOOTH = 1.0
IGNORE_INDEX = 0


def build(gp_frac=0.42):
    fp32 = mybir.dt.float32
    bf16 = mybir.dt.bfloat16
    AL = mybir.AluOpType

    nc = bacc.Bacc("TRN2", target_bir_lowering=False, debug=False)
    logits_d = nc.dram_tensor("logits", [P, NCHUNK * C], bf16, kind="ExternalInput")
    out_d = nc.dram_tensor("out", [P, P], fp32, kind="ExternalOutput")

    src = logits_d.ap().rearrange("p (b x) -> b p x", b=NBLK)

    NT = C * T               # elems per partition per block
    Tg = int(T * gp_frac) & ~7   # gpsimd's (leading) chunk share of Z

    def body(tc, pools):
        lpool, epool, zpool, small, psump, fin = pools
        acc = psump.tile([P, P], fp32)
        for blk in range(NBLK):
            Lb = lpool.tile([P, NT], bf16, tag="L")
            ndma = 4 if blk == 0 else 2
            for q in range(ndma):
                qs = slice(q * NT // ndma, (q + 1) * NT // ndma)
                nc.sync.dma_start(Lb[:, qs], src[blk, :, qs])

            E = epool.tile([P, NT], bf16, tag="E")
            y1 = zpool.tile([P, Tg * 8], bf16, tag="y1")
            y2 = zpool.tile([P, Tg * 4], bf16, tag="y2")
            y3 = zpool.tile([P, Tg * 2], bf16, tag="y3")
            Zf = small.tile([P, T], fp32, tag="Zf")
            rf = small.tile([P, T], fp32, tag="rf")
            rb = small.tile([P, T], bf16, tag="rb")

            # DMA-aligned EXPs so Z work can start early (block 0
            # quartered to shorten the pipeline fill)
            for q in range(ndma):
                qs = slice(q * NT // ndma, (q + 1) * NT // ndma)
                nc.scalar.activation(
                    E[:, qs], Lb[:, qs], mybir.ActivationFunctionType.Exp
                )

            # Z: gpsimd pair-trees the leading Tg chunks (ready after
            # EXP half 1, overlaps half 2); DVE does one contiguous
            # tensor_reduce over the rest. Zf is fp32.
            e2 = E[:].rearrange("p (k two) -> p k two", two=2)
            kg = Tg * 8
            nc.gpsimd.tensor_tensor(
                y1[:], e2[:, 0:kg, 0], e2[:, 0:kg, 1], AL.add
            )
            y1v = y1[:].rearrange("p (k two) -> p k two", two=2)
            nc.gpsimd.tensor_tensor(y2[:], y1v[:, :, 0], y1v[:, :, 1], AL.add)
            y2v = y2[:].rearrange("p (k two) -> p k two", two=2)
            nc.gpsimd.tensor_tensor(y3[:], y2v[:, :, 0], y2v[:, :, 1], AL.add)
            y3v = y3[:].rearrange("p (k two) -> p k two", two=2)
            nc.gpsimd.tensor_tensor(
                Zf[:, 0:Tg], y3v[:, :, 0], y3v[:, :, 1], AL.add
            )
            E4 = E[:].rearrange("p (j c) -> p j c", c=C)
            nc.vector.tensor_reduce(
                Zf[:, Tg:T], E4[:, Tg:T, :], mybir.AxisListType.X, AL.add
            )
            # per-share recip+cast so each share's matmuls start early;
            # tensor_scalar mult-by-1 casts at the 4x DVE rate
            nc.vector.reciprocal_approx_fast(rf[:, 0:Tg], Zf[:, 0:Tg])
            nc.vector.tensor_scalar(
                rb[:, 0:Tg], rf[:, 0:Tg], 1.0, None, AL.mult
            )
            nc.vector.reciprocal_approx_fast(rf[:, Tg:T], Zf[:, Tg:T])
            nc.vector.tensor_scalar(
                rb[:, Tg:T], rf[:, Tg:T], 1.0, None, AL.mult
            )

            # packed stats matmuls: slab s_loc of class c accumulates
            # into PSUM region acc[:, c*8:(c+1)*8]; slab 15 is dead.
            for sl in range(T // SLAB):
                creg, s_loc = divmod(sl, CREG // SLAB)
                if s_loc >= LIVE_SLABS:
                    continue
                c = blk * (T // CREG) + creg
                nc.tensor.matmul(
                    acc[:, c * SLAB : (c + 1) * SLAB],
                    E[:, sl * SLAB * C : (sl + 1) * SLAB * C],
                    rb[:, sl * SLAB : (sl + 1) * SLAB],
                    start=(s_loc == 0),
                    stop=(s_loc == LIVE_SLABS - 1),
                )
        outs = fin.tile([P, P], fp32)
        nc.vector.tensor_copy(outs[:], acc[:])
        nc.sync.dma_start(out_d.ap(), outs[:])

    with tile.TileContext(nc) as tc:
        with (
            tc.tile_pool(name="lpool", bufs=2) as lpool,
            tc.tile_pool(name="epool", bufs=2) as epool,
            tc.tile_pool(name="zpool", bufs=2) as zpool,
            tc.tile_pool(name="small", bufs=2) as small,
            tc.tile_pool(name="psum", bufs=1, space="PSUM") as psump,
            tc.tile_pool(name="fin", bufs=1) as fin,
        ):
            body(tc, (lpool, epool, zpool, small, psump, fin))
    nc.compile()
    return nc


_NC_CACHE = {}


def _get_nc():
    if "nc" not in _NC_CACHE:
        _NC_CACHE["nc"] = build()
    return _NC_CACHE["nc"]


def _prep_core(lg, tg):
    """Sort one core shard. lg [C, N] fp32, tg [N] int64 -> (dram bf16
    [P, NCHUNK*C], S_corr [C,C], I_host [C], P_host [C])."""
    order = np.argsort(tg, kind="stable")
    tsorted = tg[order]
    starts = np.searchsorted(tsorted, np.arange(C + 1))
    vmat = np.empty((NCHUNK, P), np.int64)
    leftovers = []
    n_dummy = np.zeros(C, np.int64)
    dummy_fill = order[0]  # placeholder voxel id; its logits get zeroed
    for c in range(C):
        vox = order[starts[c] : starts[c + 1]]
        nlive = min(NLIVE_C, len(vox))
        live = vox[:nlive]
        if nlive < NLIVE_C:
            n_dummy[c] = NLIVE_C - nlive
            live = np.concatenate(
                [live, np.full(NLIVE_C - nlive, dummy_fill, np.int64)]
            )
        vmat[c * CREG : c * CREG + LIVE] = live.reshape(LIVE, P)
        leftovers.append(vox[nlive:])
    left = np.concatenate(leftovers)
    ndead = NCHUNK * P - C * NLIVE_C
    pad_dead = ndead - len(left)
    assert pad_dead >= 0
    if pad_dead:
        left = np.concatenate([left, np.full(pad_dead, dummy_fill, np.int64)])
    dead_rows = np.concatenate(
        [np.arange(c * CREG + LIVE, (c + 1) * CREG) for c in range(C)]
    )
    vmat[dead_rows] = left.reshape(ndead // P, P)

    # device array [p, (chunk, class)] bf16
    arr = lg[:, vmat]                                   # [C, NCHUNK, P]
    arr = np.ascontiguousarray(arr.transpose(2, 1, 0))  # [P, NCHUNK, C]
    for c in range(C):
        nd = int(n_dummy[c])
        if nd:
            # dummy slots fill the tail of class c's live region,
            # dealt row-major over (chunk, p): zero their logits
            rows = np.arange(NLIVE_C - nd, NLIVE_C)
            chunks, ps = np.divmod(rows, P)
            arr[ps, c * CREG + chunks, :] = 0.0
    dram = arr.reshape(P, NCHUNK * C).astype(ml_dtypes.bfloat16)

    # exact uniform correction for dummy live voxels (p_hat = 1/16)
    S_corr = np.zeros((C, C), np.float64)
    for c in range(C):
        S_corr[:, c] = n_dummy[c] / 16.0

    # host softmax for leftover (dead-chunk) voxels
    I_host = np.zeros(C, np.float64)
    P_host = np.zeros(C, np.float64)
    nreal = len(left) - pad_dead
    real_left = left[:nreal]
    if nreal:
        ex = lg[:, real_left].astype(np.float64)        # [C, M]
        ex -= ex.max(axis=0, keepdims=True)
        np.exp(ex, out=ex)
        ex /= ex.sum(axis=0, keepdims=True)
        P_host = ex.sum(axis=1)
        tl = tg[real_left]
        I_host = np.bincount(
            tl, weights=ex[tl, np.arange(nreal)], minlength=C
        ).astype(np.float64)
    return dram, S_corr, I_host, P_host


def shard_inputs_meta(logits, targets):
    in_maps, meta = [], []
    for i in range(NCORES):
        b, q = divmod(i, 4)
        lg = np.ascontiguousarray(
            logits[b, :, q * DSH : (q + 1) * DSH]
        ).reshape(C, N).astype(np.float32)
        tg = np.ascontiguousarray(
            targets[b, q * DSH : (q + 1) * DSH]
        ).reshape(N).astype(np.int64)
        dram, S_corr, I_h, P_h = _prep_core(lg, tg)
        in_maps.append({"logits": dram})
        meta.append((S_corr, I_h, P_h))
    return in_maps, meta


def shard_inputs(logits, targets):
    return shard_inputs_meta(logits, targets)[0]


# extraction: S[c', c] = sum_j out[j*16 + c', c*8 + j]
_JJ = np.arange(SLAB)


def stats_from_out(out_mat):
    S = np.zeros((C, C), np.float64)
    o = out_mat.astype(np.float64)
    rows = _JJ[:, None] * C + np.arange(C)[None, :]     # [8, 16]
    for c in range(C):
        S[:, c] = o[rows, (c * SLAB + _JJ)[:, None]].sum(axis=0)
    return S


def loss_from_parts(I_b, P_b, counts):
    dice = (2.0 * I_b + SMOOTH) / (P_b + counts + SMOOTH)
    mask = np.ones(C)
    mask[IGNORE_INDEX] = 0.0
    mean_dice = (dice * mask[None, :]).sum() / (B * (C - 1))
    return np.float32(1.0 - mean_dice)


def kernel(logits, targets):
    logits = np.asarray(logits)
    targets = np.asarray(targets)
    nc = _get_nc()
    in_maps, meta = shard_inputs_meta(logits, targets)
    res = run_bass_kernel_spmd(nc, in_maps, list(range(NCORES))).results
    I_b = np.zeros((B, C), np.float64)
    P_b = np.zeros((B, C), np.float64)
    for i in range(NCORES):
        S_corr, I_h, P_h = meta[i]
        S = stats_from_out(res[i]["out"]) - S_corr
        I_b[i // 4] += np.diag(S) + I_h
        P_b[i // 4] += S.sum(axis=1) + P_h
    counts = np.stack(
        [np.bincount(targets[b].reshape(-1), minlength=C) for b in range(B)]
    ).astype(np.float64)
    return loss_from_parts(I_b, P_b, counts)


# revision 19
# speedup vs baseline: 1.1168x; 1.1168x over previous
"""Memory-efficient Dice loss on 8 Trainium2 NeuronCores.

Full inputs:
  logits  (2, 16, 64, 128, 128) fp32
  targets (2, 64, 128, 128) int64  (values 0..15)
Output: scalar fp32 loss = 1 - mean_{b, c != 0} dice[b, c].

Sharding: 8 cores over (B=2) x (D quartered into 4 slabs of 16).

Key trick: the HOST class-sorts the voxels of each core shard so that
every 128-voxel chunk is single-class, with a fixed compile-time
layout: class c owns chunks [128c, 128c+120) ("live"); the remaining
8 chunks per class region hold the leftover voxels ("dead", computed
on the host, ~6%). Voxel order is irrelevant to the dice statistics.

Device math per chunk j (128 voxels on partitions, chunk-major
[p, (j, c)] bf16 layout):
  e = exp(logits); Z = pair-tree sum over the 16-class runs;
  r = 1/Z;  S[c', class(j)] += sum_p e[p, j, c'] * r[p, j]
One packed matmul per 8-chunk slab: lhsT = E 128-col slab, rhs = the
raw 8 r-columns of that slab, out -> the class's private 8-column
PSUM region, accumulated over the class's 15 slabs. The one-hot never
exists: it is encoded in the output-region assignment; the host
extracts S[c', c] = sum_j out[j*16 + c', c*8 + j] from the final
128x128 matrix (intersection = diag, probs_sum = row sums).

Host handles: the sort/permutation, softmax of dead-chunk voxels,
exact bincount counts, and the final dice formula. If a class has
fewer than 15360 voxels its live region is padded with zero-logit
dummies whose exact uniform contribution (1/16 per entry) is
subtracted analytically.

Engine budget per core (measured rates): DMA 23us (8 MB bf16, 4
16KiB-descriptor DMAs), ACT exp 27us, DVE pair-tree+recip ~22us with
a GPSIMD level-1 share ~17us, PE 240 packed matmuls ~22us.
"""

import numpy as np
import ml_dtypes

import concourse.mybir as mybir
import concourse.tile as tile
from concourse import bacc
from concourse.bass_utils import run_bass_kernel_spmd

B, C, D, H, W = 2, 16, 64, 128, 128
P = 128            # SBUF partitions
NCORES = 8
DSH = D // 4       # d-planes per core
N = DSH * H * W    # voxels per core = 262144
NCHUNK = N // P    # 2048 chunks of 128 voxels
NBLK = 8
T = NCHUNK // NBLK # 256 chunks per block
CREG = 128         # chunks per class region
LIVE_SLABS = 15    # live slabs per class; the 16th is dead
SLAB = 8           # chunks per matmul
LIVE = LIVE_SLABS * SLAB       # 120 live chunks per class
NLIVE_C = LIVE * P             # 15360 live voxels per class

SMOOTH = 1.0
IGNORE_INDEX = 0


def build(gp_frac=0.42):
    fp32 = mybir.dt.float32
    bf16 = mybir.dt.bfloat16
    AL = mybir.AluOpType

    nc = bacc.Bacc("TRN2", target_bir_lowering=False, debug=False)
    logits_d = nc.dram_tensor("logits", [P, NCHUNK * C], bf16, kind="ExternalInput")
    out_d = nc.dram_tensor("out", [P, P], fp32, kind="ExternalOutput")

    src = logits_d.ap().rearrange("p (b x) -> b p x", b=NBLK)

    NT = C * T               # elems per partition per block
    Tg = int(T * gp_frac) & ~7   # gpsimd's (leading) chunk share of Z

    def body(tc, pools):
        lpool, epool, zpool, small, psump, fin = pools
        acc = psump.tile([P, P], fp32)
        for blk in range(NBLK):
            Lb = lpool.tile([P, NT], bf16, tag="L")
            nc.sync.dma_start(Lb[:, 0 : NT // 2], src[blk, :, 0 : NT // 2])
            nc.sync.dma_start(Lb[:, NT // 2 :], src[blk, :, NT // 2 :])

            E = epool.tile([P, NT], bf16, tag="E")
            y1 = zpool.tile([P, Tg * 8], bf16, tag="y1")
            y2 = zpool.tile([P, Tg * 4], bf16, tag="y2")
            y3 = zpool.tile([P, Tg * 2], bf16, tag="y3")
            Zf = small.tile([P, T], fp32, tag="Zf")
            rf = small.tile([P, T], fp32, tag="rf")
            rb = small.tile([P, T], bf16, tag="rb")

            # split DMA-half-aligned EXPs so Z work can start early
            nc.scalar.activation(
                E[:, 0 : NT // 2], Lb[:, 0 : NT // 2],
                mybir.ActivationFunctionType.Exp,
            )
            nc.scalar.activation(
                E[:, NT // 2 :], Lb[:, NT // 2 :],
                mybir.ActivationFunctionType.Exp,
            )

            # Z: gpsimd pair-trees the leading Tg chunks (ready after
            # EXP half 1, overlaps half 2); DVE does one contiguous
            # tensor_reduce over the rest. Zf is fp32.
            e2 = E[:].rearrange("p (k two) -> p k two", two=2)
            kg = Tg * 8
            nc.gpsimd.tensor_tensor(
                y1[:], e2[:, 0:kg, 0], e2[:, 0:kg, 1], AL.add
            )
            y1v = y1[:].rearrange("p (k two) -> p k two", two=2)
            nc.gpsimd.tensor_tensor(y2[:], y1v[:, :, 0], y1v[:, :, 1], AL.add)
            y2v = y2[:].rearrange("p (k two) -> p k two", two=2)
            nc.gpsimd.tensor_tensor(y3[:], y2v[:, :, 0], y2v[:, :, 1], AL.add)
            y3v = y3[:].rearrange("p (k two) -> p k two", two=2)
            nc.gpsimd.tensor_tensor(
                Zf[:, 0:Tg], y3v[:, :, 0], y3v[:, :, 1], AL.add
            )
            E4 = E[:].rearrange("p (j c) -> p j c", c=C)
            nc.vector.tensor_reduce(
                Zf[:, Tg:T], E4[:, Tg:T, :], mybir.AxisListType.X, AL.add
            )
            nc.vector.reciprocal_approx_fast(rf[:], Zf[:])
            nc.vector.tensor_scalar(rb[:], rf[:], 1.0, None, AL.mult)

            # packed stats matmuls: slab s_loc of class c accumulates
            # into PSUM region acc[:, c*8:(c+1)*8]; slab 15 is dead.
            for sl in range(T // SLAB):
                creg, s_loc = divmod(sl, CREG // SLAB)
                if s_loc >= LIVE_SLABS:
                    continue
                c = blk * (T // CREG) + creg
                nc.tensor.matmul(
                    acc[:, c * SLAB : (c + 1) * SLAB],
                    E[:, sl * SLAB * C : (sl + 1) * SLAB * C],
                    rb[:, sl * SLAB : (sl + 1) * SLAB],
                    start=(s_loc == 0),
                    stop=(s_loc == LIVE_SLABS - 1),
                )
        outs = fin.tile([P, P], fp32)
        nc.vector.tensor_copy(outs[:], acc[:])
        nc.sync.dma_start(out_d.ap(), outs[:])

    with tile.TileContext(nc) as tc:
        with (
            tc.tile_pool(name="lpool", bufs=2) as lpool,
            tc.tile_pool(name="epool", bufs=2) as epool,
            tc.tile_pool(name="zpool", bufs=2) as zpool,
            tc.tile_pool(name="small", bufs=2) as small,
            tc.tile_pool(name="psum", bufs=1, space="PSUM") as psump,
            tc.tile_pool(name="fin", bufs=1) as fin,
        ):
            body(tc, (lpool, epool, zpool, small, psump, fin))
    nc.compile()
    return nc


_NC_CACHE = {}


def _get_nc():
    if "nc" not in _NC_CACHE:
        _NC_CACHE["nc"] = build()
    return _NC_CACHE["nc"]


def _prep_core(lg, tg):
    """Sort one core shard. lg [C, N] fp32, tg [N] int64 -> (dram bf16
    [P, NCHUNK*C], S_corr [C,C], I_host [C], P_host [C])."""
    order = np.argsort(tg, kind="stable")
    tsorted = tg[order]
    starts = np.searchsorted(tsorted, np.arange(C + 1))
    vmat = np.empty((NCHUNK, P), np.int64)
    leftovers = []
    n_dummy = np.zeros(C, np.int64)
    dummy_fill = order[0]  # placeholder voxel id; its logits get zeroed
    for c in range(C):
        vox = order[starts[c] : starts[c + 1]]
        nlive = min(NLIVE_C, len(vox))
        live = vox[:nlive]
        if nlive < NLIVE_C:
            n_dummy[c] = NLIVE_C - nlive
            live = np.concatenate(
                [live, np.full(NLIVE_C - nlive, dummy_fill, np.int64)]
            )
        vmat[c * CREG : c * CREG + LIVE] = live.reshape(LIVE, P)
        leftovers.append(vox[nlive:])
    left = np.concatenate(leftovers)
    ndead = NCHUNK * P - C * NLIVE_C
    pad_dead = ndead - len(left)
    assert pad_dead >= 0
    if pad_dead:
        left = np.concatenate([left, np.full(pad_dead, dummy_fill, np.int64)])
    dead_rows = np.concatenate(
        [np.arange(c * CREG + LIVE, (c + 1) * CREG) for c in range(C)]
    )
    vmat[dead_rows] = left.reshape(ndead // P, P)

    # device array [p, (chunk, class)] bf16
    arr = lg[:, vmat]                                   # [C, NCHUNK, P]
    arr = np.ascontiguousarray(arr.transpose(2, 1, 0))  # [P, NCHUNK, C]
    for c in range(C):
        nd = int(n_dummy[c])
        if nd:
            # dummy slots fill the tail of class c's live region,
            # dealt row-major over (chunk, p): zero their logits
            rows = np.arange(NLIVE_C - nd, NLIVE_C)
            chunks, ps = np.divmod(rows, P)
            arr[ps, c * CREG + chunks, :] = 0.0
    dram = arr.reshape(P, NCHUNK * C).astype(ml_dtypes.bfloat16)

    # exact uniform correction for dummy live voxels (p_hat = 1/16)
    S_corr = np.zeros((C, C), np.float64)
    for c in range(C):
        S_corr[:, c] = n_dummy[c] / 16.0

    # host softmax for leftover (dead-chunk) voxels
    I_host = np.zeros(C, np.float64)
    P_host = np.zeros(C, np.float64)
    nreal = len(left) - pad_dead
    real_left = left[:nreal]
    if nreal:
        ex = lg[:, real_left].astype(np.float64)        # [C, M]
        ex -= ex.max(axis=0, keepdims=True)
        np.exp(ex, out=ex)
        ex /= ex.sum(axis=0, keepdims=True)
        P_host = ex.sum(axis=1)
        tl = tg[real_left]
        I_host = np.bincount(
            tl, weights=ex[tl, np.arange(nreal)], minlength=C
        ).astype(np.float64)
    return dram, S_corr, I_host, P_host


def shard_inputs_meta(logits, targets):
    in_maps, meta = [], []
    for i in range(NCORES):
        b, q = divmod(i, 4)
        lg = np.ascontiguousarray(
            logits[b, :, q * DSH : (q + 1) * DSH]
        ).reshape(C, N).astype(np.float32)
        tg = np.ascontiguousarray(
            targets[b, q * DSH : (q + 1) * DSH]
        ).reshape(N).astype(np.int64)
        dram, S_corr, I_h, P_h = _prep_core(lg, tg)
        in_maps.append({"logits": dram})
        meta.append((S_corr, I_h, P_h))
    return in_maps, meta


def shard_inputs(logits, targets):
    return shard_inputs_meta(logits, targets)[0]


# extraction: S[c', c] = sum_j out[j*16 + c', c*8 + j]
_JJ = np.arange(SLAB)


def stats_from_out(out_mat):
    S = np.zeros((C, C), np.float64)
    o = out_mat.astype(np.float64)
    rows = _JJ[:, None] * C + np.arange(C)[None, :]     # [8, 16]
    for c in range(C):
        S[:, c] = o[rows, (c * SLAB + _JJ)[:, None]].sum(axis=0)
    return S


def loss_from_parts(I_b, P_b, counts):
    dice = (2.0 * I_b + SMOOTH) / (P_b + counts + SMOOTH)
    mask = np.ones(C)
    mask[IGNORE_INDEX] = 0.0
    mean_dice = (dice * mask[None, :]).sum() / (B * (C - 1))
    return np.float32(1.0 - mean_dice)


def kernel(logits, targets):
    logits = np.asarray(logits)
    targets = np.asarray(targets)
    nc = _get_nc()
    in_maps, meta = shard_inputs_meta(logits, targets)
    res = run_bass_kernel_spmd(nc, in_maps, list(range(NCORES))).results
    I_b = np.zeros((B, C), np.float64)
    P_b = np.zeros((B, C), np.float64)
    for i in range(NCORES):
        S_corr, I_h, P_h = meta[i]
        S = stats_from_out(res[i]["out"]) - S_corr
        I_b[i // 4] += np.diag(S) + I_h
        P_b[i // 4] += S.sum(axis=1) + P_h
    counts = np.stack(
        [np.bincount(targets[b].reshape(-1), minlength=C) for b in range(B)]
    ).astype(np.float64)
    return loss_from_parts(I_b, P_b, counts)
